# revision 11
# baseline (speedup 1.0000x reference)
"""Bilinear edge predictor on 8 Trainium2 NeuronCores.

scores[e, c] = h[src[e]] @ W[c] @ h[dst[e]] + b[c]

Sharding: edges globally sorted by dst and split into 8 equal groups
(dst-range sharding); h, W, b replicated.  Host unsorts the scores.

Per-core pipeline ([feature, edge] layout, 896-edge chunks):
  - src side: dma_gather(transpose=True) pulls huT = h[src] columns
    [128 feat, 896 edges] bf16 from per-phase HBM tables with int16
    indices (host renumbers per phase).  Gathers spread over 4 SWDGE
    queues (queue_num re-keyed post-schedule to the DMASW sem lane).
  - dst side: edges are dst-sorted, so each chunk touches <=128 unique
    dst nodes.  A static DMA loads hvuT [128k, 128u] (per-chunk block
    of the transposed dedup table).  DVE forms column diffs dhvT fp16;
    PE computes DWgT[u, c*128f] = dhvT^T @ W[c]; DVE builds a step
    matrix step[u, e] = (e >= s_u) from per-partition boundaries; PE
    expands WhvT[f, e] = sum_u DWgT[u, f] * step[u, e]  (telescoping:
    partial sums of diffs reconstruct W @ h[dst[e]]).
  - DVE: prodT[f, e] = huT * WhvT (3 classes via ACT PSUM->SBUF bf16
    copy + DVE 2x; 1 class fused from PSUM).
  - PE "selector-ones" matmul reduces over f into 20 PSUM rows
    (5 chunks x 4 classes); ACT exits scores with fused bias add.
"""

import numpy as np

N_NODES = 40000
H = 128
C = 4
E = 640000
N_CORES = 8
P = 128

E_CORE = E // N_CORES          # 80000
CHUNK = 896                    # edges per chunk (transpose ring limit)
NA = 512                       # half A columns (PSUM bank)
NB = CHUNK - NA                # half B columns (384)
SUPER = 5                      # chunks per score-accumulation supertile
NCHUNK = 90                    # chunks per core (E_CORE/CHUNK rounded up, %SUPER==0)
PHASES = 5                     # src gather phases
CPP = NCHUNK // PHASES         # chunks per phase (19)
TAB = 32768                    # rows per src phase table (int16 range)
ICOLS = CHUNK // 16            # idx columns per chunk (56)
E_PAD = NCHUNK * CHUNK         # 85120 padded edge slots per core

_kernel_cache = {}
_last_run = {}


def _build():
    import concourse.bacc as bacc
    import concourse.tile as tile
    from concourse import mybir
    from concourse import library_config

    nc = bacc.Bacc(None, target_bir_lowering=False, debug=False,
                   num_swdge_queues=4)
    with tile.TileContext(nc) as tc:
        with tc.tile_pool(name="dram", bufs=1, space="DRAM") as dram:
            htab_d = dram.tile([PHASES, TAB, H], mybir.dt.bfloat16,
                               kind="ExternalInput", name="htab", uniquify=False)
            dstt_d = dram.tile([H, NCHUNK * P], mybir.dt.bfloat16,
                               kind="ExternalInput", name="dstt", uniquify=False)
            wt_d = dram.tile([H, C, H], mybir.dt.float16,
                             kind="ExternalInput", name="wt", uniquify=False)
            sel_d = dram.tile([P, SUPER * C, SUPER * C], mybir.dt.bfloat16,
                              kind="ExternalInput", name="sel", uniquify=False)
            bias_d = dram.tile([SUPER * C, 1], mybir.dt.float32,
                               kind="ExternalInput", name="bias", uniquify=False)
            iota_d = dram.tile([P, CHUNK], mybir.dt.float16,
                               kind="ExternalInput", name="iota", uniquify=False)
            stab_d = dram.tile([P, NCHUNK], mybir.dt.float16,
                               kind="ExternalInput", name="stab", uniquify=False)
            src_d = dram.tile([P, NCHUNK * ICOLS], mybir.dt.int16,
                              kind="ExternalInput", name="srcx", uniquify=False)
            out_d = dram.tile([NCHUNK, C, CHUNK], mybir.dt.float32,
                              kind="ExternalOutput", name="scores", uniquify=False)

            with (
                tc.tile_pool(name="const", bufs=1) as cpool,
                tc.tile_pool(name="gat", bufs=6) as gpool,
                tc.tile_pool(name="hvu", bufs=4) as hvpool,
                tc.tile_pool(name="dwg", bufs=3) as dwgpool,
                tc.tile_pool(name="stp", bufs=3) as stpool,
                tc.tile_pool(name="whvp", bufs=6) as wpool,
                tc.tile_pool(name="pr", bufs=6) as prpool,
                tc.tile_pool(name="sco", bufs=2) as scpool,
                tc.tile_pool(name="ps_g", bufs=2, space="PSUM") as ps_g,
                tc.tile_pool(name="ps_w", bufs=2, space="PSUM") as ps_w,
                tc.tile_pool(name="ps_s", bufs=1, space="PSUM") as ps_s,
            ):
                wt_sb = cpool.tile([H, C, H], mybir.dt.float16, name="wt_sb")
                nc.sync.dma_start(out=wt_sb[:], in_=wt_d[:])
                sel_sb = cpool.tile([P, SUPER * C, SUPER * C], mybir.dt.bfloat16,
                                    name="sel_sb")
                nc.sync.dma_start(out=sel_sb[:], in_=sel_d[:])
                bias_sb = cpool.tile([SUPER * C, 1], mybir.dt.float32,
                                     name="bias_sb")
                nc.sync.dma_start(out=bias_sb[:], in_=bias_d[:])
                iota_sb = cpool.tile([P, CHUNK], mybir.dt.float16, name="iota_sb")
                nc.sync.dma_start(out=iota_sb[:], in_=iota_d[:])
                stab_sb = cpool.tile([P, NCHUNK], mybir.dt.float16, name="stab_sb")
                nc.sync.dma_start(out=stab_sb[:], in_=stab_d[:])
                src_sb = cpool.tile([P, NCHUNK * ICOLS], mybir.dt.int16,
                                    name="src_sb")
                nc.sync.dma_start(out=src_sb[:], in_=src_d[:])

                nc.gpsimd.load_library(library_config.mlp)

                for s0 in range(0, NCHUNK, SUPER):
                    sca = ps_s.tile([SUPER * C, NA], mybir.dt.float32,
                                    name="sca", tag="sca")
                    scb = ps_s.tile([SUPER * C, NB], mybir.dt.float32,
                                    name="scb", tag="scb")
                    for ci in range(SUPER):
                        ch = s0 + ci
                        ph = ch // CPP
                        isl = slice(ch * ICOLS, (ch + 1) * ICOLS)
                        huT = gpool.tile([P, 1, CHUNK], mybir.dt.bfloat16,
                                         name="huT", tag="huT")
                        nc.gpsimd.dma_gather(huT[:], htab_d[ph], src_sb[:, isl],
                                             CHUNK, CHUNK, H, transpose=True)

                        hvu = hvpool.tile([P, P], mybir.dt.bfloat16,
                                          name="hvu", tag="hvu")
                        nc.sync.dma_start(
                            out=hvu[:], in_=dstt_d[:, ch * P:(ch + 1) * P])
                        dhv = hvpool.tile([P, P], mybir.dt.float16,
                                          name="dhv", tag="dhv")
                        nc.vector.tensor_copy(out=dhv[:, 0:1], in_=hvu[:, 0:1])
                        nc.vector.tensor_tensor(
                            out=dhv[:, 1:P],
                            in0=hvu[:, 1:P],
                            in1=hvu[:, 0:P - 1],
                            op=mybir.AluOpType.subtract,
                        )

                        dwg_ps = ps_g.tile([P, C * H], mybir.dt.float32,
                                           name="dwg_ps", tag="dwg_ps")
                        nc.tensor.matmul(
                            out=dwg_ps[:],
                            lhsT=dhv[:],
                            rhs=wt_sb[:, :, :],
                            start=True, stop=True,
                        )
                        dwg_sb = dwgpool.tile([P, C, H], mybir.dt.float16,
                                              name="dwg_sb", tag="dwg_sb")
                        nc.scalar.copy(out=dwg_sb[:], in_=dwg_ps[:])

                        step = stpool.tile([P, CHUNK], mybir.dt.float16,
                                           name="step", tag="step")
                        nc.vector.tensor_tensor(
                            out=step[:],
                            in0=iota_sb[:],
                            in1=stab_sb[:, ch:ch + 1].to_broadcast([P, CHUNK]),
                            op=mybir.AluOpType.is_ge,
                        )

                        for c in range(C):
                            whv_ps = ps_w.tile([P, CHUNK], mybir.dt.float32,
                                               name="whv_ps", tag="whv_ps",
                                               padded_shape=[P, 1024])
                            nc.tensor.matmul(
                                out=whv_ps[:, :NA],
                                lhsT=dwg_sb[:, c, :],
                                rhs=step[:, :NA],
                                start=True, stop=True,
                            )
                            nc.tensor.matmul(
                                out=whv_ps[:, NA:],
                                lhsT=dwg_sb[:, c, :],
                                rhs=step[:, NA:],
                                start=True, stop=True,
                            )
                            prod = prpool.tile([P, CHUNK], mybir.dt.bfloat16,
                                               name="prod", tag="prod")
                            if c < C - 1:
                                # ACT exits PSUM->SBUF bf16; DVE/Pool muls at 2x
                                whv_sb = wpool.tile([P, CHUNK], mybir.dt.bfloat16,
                                                    name="whv_sb", tag="whv_sb")
                                nc.scalar.copy(out=whv_sb[:], in_=whv_ps[:])
                                mul_eng = nc.gpsimd if c == 0 else nc.vector
                                mul_eng.tensor_tensor(
                                    out=prod[:],
                                    in0=huT[:, 0, :],
                                    in1=whv_sb[:],
                                    op=mybir.AluOpType.mult,
                                )
                            else:
                                # DVE mul straight from PSUM (fuses exit)
                                nc.vector.tensor_tensor(
                                    out=prod[:],
                                    in0=huT[:, 0, :],
                                    in1=whv_ps[:],
                                    op=mybir.AluOpType.mult,
                                )
                            r = ci * C + c
                            nc.tensor.matmul(
                                out=sca[:],
                                lhsT=sel_sb[:, r, :],
                                rhs=prod[:, :NA],
                                start=(r == 0), stop=(r == SUPER * C - 1),
                                skip_group_check=True,
                            )
                            nc.tensor.matmul(
                                out=scb[:],
                                lhsT=sel_sb[:, r, :],
                                rhs=prod[:, NA:],
                                start=(r == 0), stop=(r == SUPER * C - 1),
                                skip_group_check=True,
                            )
                    sc_sb = scpool.tile([SUPER * C, CHUNK], mybir.dt.float32,
                                        name="sc_sb", tag="sc_sb")
                    nc.scalar.activation(
                        out=sc_sb[:, :NA], in_=sca[:],
                        func=mybir.ActivationFunctionType.Identity,
                        bias=bias_sb[:], scale=1.0,
                    )
                    nc.scalar.activation(
                        out=sc_sb[:, NA:], in_=scb[:],
                        func=mybir.ActivationFunctionType.Identity,
                        bias=bias_sb[:], scale=1.0,
                    )
                    for ci in range(SUPER):
                        nc.sync.dma_start(
                            out=out_d[s0 + ci],
                            in_=sc_sb[ci * C:(ci + 1) * C, :],
                        )
    # Tile rotates each Pool-engine DMA over 8 DMASW sem lanes in scheduled
    # order; a sem lane must stay on one SWDGE queue, so derive queue_num
    # from the assigned lane (lane % 4) to spread desc-gen over 4 queues.
    from concourse.tile_scheduler import PROC_NAME_TO_IDX
    from concourse import mybir as _mb
    idx_to_name = {v: k for k, v in PROC_NAME_TO_IDX.items()}
    for inst in nc.inst_map.values():
        if isinstance(inst, _mb.InstDMAGatherAnt):
            proc_name = idx_to_name[inst.bass_scheduled_proc]
            assert proc_name.startswith("DMASW"), proc_name
            inst.queue_num = int(proc_name[len("DMASW"):]) % 4
    nc.compile()
    return nc


def _get_kernel():
    if "nc" not in _kernel_cache:
        _kernel_cache["nc"] = _build()
    return _kernel_cache["nc"]


def _prep_core(hbf, src_c, dst_c):
    """Per-core arrays.  src_c/dst_c are the core's E_CORE edges with
    dst_c sorted ascending.  Returns htab, src16, dstT, stab."""
    pe = E_PAD - len(src_c)
    s_p = np.concatenate([src_c, np.zeros(pe, src_c.dtype)])
    d_p = np.concatenate([dst_c, np.full(pe, dst_c[-1], dst_c.dtype)])

    htab = np.zeros((PHASES, TAB, H), hbf.dtype)
    src16 = np.zeros((P, NCHUNK * ICOLS), np.int16)
    pedges = CPP * CHUNK
    for ph in range(PHASES):
        ids = s_p[ph * pedges:(ph + 1) * pedges]
        uniq, inv = np.unique(ids, return_inverse=True)
        assert len(uniq) <= TAB, len(uniq)
        htab[ph, :len(uniq)] = hbf[uniq]
        blk = inv.astype(np.int16).reshape(CPP, ICOLS, 16)
        row16 = blk.transpose(2, 0, 1).reshape(16, CPP * ICOLS)
        src16[:, ph * CPP * ICOLS:(ph + 1) * CPP * ICOLS] = np.tile(row16, (8, 1))

    dstT = np.zeros((H, NCHUNK * P), hbf.dtype)
    stab = np.full((P, NCHUNK), CHUNK, np.float16)
    for ci in range(NCHUNK):
        d = d_p[ci * CHUNK:(ci + 1) * CHUNK]
        uniq, colid = np.unique(d, return_inverse=True)
        assert len(uniq) <= P, len(uniq)
        dstT[:, ci * P:ci * P + len(uniq)] = hbf[uniq].T
        stab[:, ci] = np.searchsorted(colid, np.arange(P)).astype(np.float16)
    return htab, src16, dstT, stab


def kernel(h, W, b, src, dst):
    import ml_dtypes
    from concourse.bass_utils import run_bass_kernel_spmd

    h = np.ascontiguousarray(np.asarray(h, dtype=np.float32))
    W = np.asarray(W, dtype=np.float32)
    b = np.asarray(b, dtype=np.float32)
    src = np.asarray(src)
    dst = np.asarray(dst)

    hbf = h.astype(ml_dtypes.bfloat16)
    # wt[k, c, f] = W[c, f, k], fp16 for the low-error diff chain
    wt = np.ascontiguousarray(W.transpose(2, 0, 1)).astype(np.float16)
    sel = np.zeros((P, SUPER * C, SUPER * C), np.float32)
    for r in range(SUPER * C):
        sel[:, r, r] = 1.0
    sel = sel.astype(ml_dtypes.bfloat16)
    bias = np.ascontiguousarray(
        np.tile(b[None, :], (SUPER, 1)).reshape(SUPER * C, 1)).astype(np.float32)
    iota = np.tile(np.arange(CHUNK, dtype=np.float16)[None, :], (P, 1))

    order = np.argsort(dst, kind="stable")
    in_maps = []
    for i in range(N_CORES):
        e_idx = order[i * E_CORE:(i + 1) * E_CORE]
        htab, src16, dstT, stab = _prep_core(hbf, src[e_idx], dst[e_idx])
        in_maps.append({
            "htab": htab, "dstt": dstT, "wt": wt, "sel": sel, "bias": bias,
            "iota": iota, "stab": stab, "srcx": src16,
        })

    nc = _get_kernel()
    _last_run["nc"] = nc
    _last_run["in_maps"] = in_maps
    res = run_bass_kernel_spmd(nc, in_maps, core_ids=list(range(N_CORES)))

    out = np.empty((E, C), np.float32)
    for i in range(N_CORES):
        sc = res.results[i]["scores"]              # [NCHUNK, C, CHUNK]
        slots = sc.transpose(0, 2, 1).reshape(E_PAD, C)
        out[order[i * E_CORE:(i + 1) * E_CORE]] = slots[:E_CORE]
    return out


# revision 12
# speedup vs baseline: 4.5308x; 4.5308x over previous
"""Bilinear edge predictor on 8 Trainium2 NeuronCores.

scores[e, c] = h[src[e]] @ W[c] @ h[dst[e]] + b[c]

Sharding: edges globally sorted by dst and split into 8 equal groups
(dst-range sharding); h, W, b replicated.  Host unsorts the scores.

Per-core pipeline ([feature, edge] layout, 896-edge chunks):
  - src side: dma_gather(transpose=True) pulls huT = h[src] columns
    [128 feat, 896 edges] bf16 from per-phase HBM tables with int16
    indices (host renumbers per phase).  Gathers spread over 4 SWDGE
    queues (queue_num re-keyed post-schedule to the DMASW sem lane).
  - dst side: edges are dst-sorted, so each chunk touches <=128 unique
    dst nodes.  A static DMA loads hvuT [128k, 128u] (per-chunk block
    of the transposed dedup table).  DVE forms column diffs dhvT fp16;
    PE computes DWgT[u, c*128f] = dhvT^T @ W[c]; DVE builds a step
    matrix step[u, e] = (e >= s_u) from per-partition boundaries; PE
    expands WhvT[f, e] = sum_u DWgT[u, f] * step[u, e]  (telescoping:
    partial sums of diffs reconstruct W @ h[dst[e]]).
  - DVE: prodT[f, e] = huT * WhvT (3 classes via ACT PSUM->SBUF bf16
    copy + DVE 2x; 1 class fused from PSUM).
  - PE "selector-ones" matmul reduces over f into 20 PSUM rows
    (5 chunks x 4 classes); ACT exits scores with fused bias add.
"""

import numpy as np

N_NODES = 40000
H = 128
C = 4
E = 640000
N_CORES = 8
P = 128

E_CORE = E // N_CORES          # 80000
CHUNK = 896                    # edges per chunk (transpose ring limit)
NA = 512                       # half A columns (PSUM bank)
NB = CHUNK - NA                # half B columns (384)
SUPER = 5                      # chunks per score-accumulation supertile
NCHUNK = 90                    # chunks per core (E_CORE/CHUNK rounded up, %SUPER==0)
PHASES = 5                     # src gather phases
CPP = NCHUNK // PHASES         # chunks per phase (19)
TAB = 32768                    # rows per src phase table (int16 range)
ICOLS = CHUNK // 16            # idx columns per chunk (56)
E_PAD = NCHUNK * CHUNK         # 85120 padded edge slots per core

_kernel_cache = {}
_last_run = {}


def _build():
    import concourse.bacc as bacc
    import concourse.tile as tile
    from concourse import mybir
    from concourse import library_config

    nc = bacc.Bacc(None, target_bir_lowering=False, debug=False,
                   num_swdge_queues=4)
    with tile.TileContext(nc) as tc:
        with tc.tile_pool(name="dram", bufs=1, space="DRAM") as dram:
            htab_d = dram.tile([PHASES, TAB, H], mybir.dt.bfloat16,
                               kind="ExternalInput", name="htab", uniquify=False)
            dstt_d = dram.tile([H, NCHUNK * P], mybir.dt.bfloat16,
                               kind="ExternalInput", name="dstt", uniquify=False)
            wt_d = dram.tile([H, C, H], mybir.dt.float16,
                             kind="ExternalInput", name="wt", uniquify=False)
            sel_d = dram.tile([P, SUPER * C, SUPER * C], mybir.dt.bfloat16,
                              kind="ExternalInput", name="sel", uniquify=False)
            bias_d = dram.tile([SUPER * C, 1], mybir.dt.float32,
                               kind="ExternalInput", name="bias", uniquify=False)
            iota_d = dram.tile([P, CHUNK], mybir.dt.float16,
                               kind="ExternalInput", name="iota", uniquify=False)
            stab_d = dram.tile([P, NCHUNK], mybir.dt.float16,
                               kind="ExternalInput", name="stab", uniquify=False)
            src_d = dram.tile([P, NCHUNK * ICOLS], mybir.dt.int16,
                              kind="ExternalInput", name="srcx", uniquify=False)
            out_d = dram.tile([NCHUNK, C, CHUNK], mybir.dt.float32,
                              kind="ExternalOutput", name="scores", uniquify=False)

            with (
                tc.tile_pool(name="const", bufs=1) as cpool,
                tc.tile_pool(name="gat", bufs=6) as gpool,
                tc.tile_pool(name="hvu", bufs=4) as hvpool,
                tc.tile_pool(name="dwg", bufs=3) as dwgpool,
                tc.tile_pool(name="stp", bufs=3) as stpool,
                tc.tile_pool(name="whvp", bufs=6) as wpool,
                tc.tile_pool(name="pr", bufs=6) as prpool,
                tc.tile_pool(name="sco", bufs=2) as scpool,
                tc.tile_pool(name="ps_g", bufs=2, space="PSUM") as ps_g,
                tc.tile_pool(name="ps_w", bufs=2, space="PSUM") as ps_w,
                tc.tile_pool(name="ps_s", bufs=1, space="PSUM") as ps_s,
            ):
                wt_sb = cpool.tile([H, C, H], mybir.dt.float16, name="wt_sb")
                nc.sync.dma_start(out=wt_sb[:], in_=wt_d[:])
                sel_sb = cpool.tile([P, SUPER * C, SUPER * C], mybir.dt.bfloat16,
                                    name="sel_sb")
                nc.sync.dma_start(out=sel_sb[:], in_=sel_d[:])
                bias_sb = cpool.tile([SUPER * C, 1], mybir.dt.float32,
                                     name="bias_sb")
                nc.sync.dma_start(out=bias_sb[:], in_=bias_d[:])
                iota_sb = cpool.tile([P, CHUNK], mybir.dt.float16, name="iota_sb")
                nc.sync.dma_start(out=iota_sb[:], in_=iota_d[:])
                stab_sb = cpool.tile([P, NCHUNK], mybir.dt.float16, name="stab_sb")
                nc.sync.dma_start(out=stab_sb[:], in_=stab_d[:])
                src_sb = cpool.tile([P, NCHUNK * ICOLS], mybir.dt.int16,
                                    name="src_sb")
                nc.sync.dma_start(out=src_sb[:], in_=src_d[:])

                nc.gpsimd.load_library(library_config.mlp)

                for s0 in range(0, NCHUNK, SUPER):
                    sca = ps_s.tile([SUPER * C, NA], mybir.dt.float32,
                                    name="sca", tag="sca")
                    scb = ps_s.tile([SUPER * C, NB], mybir.dt.float32,
                                    name="scb", tag="scb")
                    for ci in range(SUPER):
                        ch = s0 + ci
                        ph = ch // CPP
                        isl = slice(ch * ICOLS, (ch + 1) * ICOLS)
                        huT = gpool.tile([P, 1, CHUNK], mybir.dt.bfloat16,
                                         name="huT", tag="huT")
                        nc.gpsimd.dma_gather(huT[:], htab_d[ph], src_sb[:, isl],
                                             CHUNK, CHUNK, H, transpose=True)

                        hvu = hvpool.tile([P, P], mybir.dt.bfloat16,
                                          name="hvu", tag="hvu")
                        nc.sync.dma_start(
                            out=hvu[:], in_=dstt_d[:, ch * P:(ch + 1) * P])
                        dhv = hvpool.tile([P, P], mybir.dt.float16,
                                          name="dhv", tag="dhv")
                        nc.vector.tensor_copy(out=dhv[:, 0:1], in_=hvu[:, 0:1])
                        nc.vector.tensor_tensor(
                            out=dhv[:, 1:P],
                            in0=hvu[:, 1:P],
                            in1=hvu[:, 0:P - 1],
                            op=mybir.AluOpType.subtract,
                        )

                        dwg_ps = ps_g.tile([P, C * H], mybir.dt.float32,
                                           name="dwg_ps", tag="dwg_ps")
                        nc.tensor.matmul(
                            out=dwg_ps[:],
                            lhsT=dhv[:],
                            rhs=wt_sb[:, :, :],
                            start=True, stop=True,
                        )
                        dwg_sb = dwgpool.tile([P, C, H], mybir.dt.float16,
                                              name="dwg_sb", tag="dwg_sb")
                        nc.scalar.copy(out=dwg_sb[:], in_=dwg_ps[:])

                        step = stpool.tile([P, CHUNK], mybir.dt.float16,
                                           name="step", tag="step")
                        nc.vector.tensor_tensor(
                            out=step[:],
                            in0=iota_sb[:],
                            in1=stab_sb[:, ch:ch + 1].to_broadcast([P, CHUNK]),
                            op=mybir.AluOpType.is_ge,
                        )

                        for c in range(C):
                            whv_ps = ps_w.tile([P, CHUNK], mybir.dt.float32,
                                               name="whv_ps", tag="whv_ps",
                                               padded_shape=[P, 1024])
                            nc.tensor.matmul(
                                out=whv_ps[:, :NA],
                                lhsT=dwg_sb[:, c, :],
                                rhs=step[:, :NA],
                                start=True, stop=True,
                            )
                            nc.tensor.matmul(
                                out=whv_ps[:, NA:],
                                lhsT=dwg_sb[:, c, :],
                                rhs=step[:, NA:],
                                start=True, stop=True,
                            )
                            prod = prpool.tile([P, CHUNK], mybir.dt.bfloat16,
                                               name="prod", tag="prod")
                            if c < C - 1:
                                # ACT exits PSUM->SBUF bf16; DVE/Pool muls at 2x
                                whv_sb = wpool.tile([P, CHUNK], mybir.dt.bfloat16,
                                                    name="whv_sb", tag="whv_sb")
                                nc.scalar.copy(out=whv_sb[:], in_=whv_ps[:])
                                nc.vector.tensor_tensor(
                                    out=prod[:],
                                    in0=huT[:, 0, :],
                                    in1=whv_sb[:],
                                    op=mybir.AluOpType.mult,
                                )
                            else:
                                # DVE mul straight from PSUM (fuses exit)
                                nc.vector.tensor_tensor(
                                    out=prod[:],
                                    in0=huT[:, 0, :],
                                    in1=whv_ps[:],
                                    op=mybir.AluOpType.mult,
                                )
                            r = ci * C + c
                            nc.tensor.matmul(
                                out=sca[:],
                                lhsT=sel_sb[:, r, :],
                                rhs=prod[:, :NA],
                                start=(r == 0), stop=(r == SUPER * C - 1),
                                skip_group_check=True,
                            )
                            nc.tensor.matmul(
                                out=scb[:],
                                lhsT=sel_sb[:, r, :],
                                rhs=prod[:, NA:],
                                start=(r == 0), stop=(r == SUPER * C - 1),
                                skip_group_check=True,
                            )
                    sc_sb = scpool.tile([SUPER * C, CHUNK], mybir.dt.float32,
                                        name="sc_sb", tag="sc_sb")
                    nc.scalar.activation(
                        out=sc_sb[:, :NA], in_=sca[:],
                        func=mybir.ActivationFunctionType.Identity,
                        bias=bias_sb[:], scale=1.0,
                    )
                    nc.scalar.activation(
                        out=sc_sb[:, NA:], in_=scb[:],
                        func=mybir.ActivationFunctionType.Identity,
                        bias=bias_sb[:], scale=1.0,
                    )
                    for ci in range(SUPER):
                        nc.sync.dma_start(
                            out=out_d[s0 + ci],
                            in_=sc_sb[ci * C:(ci + 1) * C, :],
                        )
    # Tile rotates each Pool-engine DMA over 8 DMASW sem lanes in scheduled
    # order; a sem lane must stay on one SWDGE queue, so derive queue_num
    # from the assigned lane (lane % 4) to spread desc-gen over 4 queues.
    from concourse.tile_scheduler import PROC_NAME_TO_IDX
    from concourse import mybir as _mb
    idx_to_name = {v: k for k, v in PROC_NAME_TO_IDX.items()}
    for inst in nc.inst_map.values():
        if isinstance(inst, _mb.InstDMAGatherAnt):
            proc_name = idx_to_name[inst.bass_scheduled_proc]
            assert proc_name.startswith("DMASW"), proc_name
            inst.queue_num = int(proc_name[len("DMASW"):]) % 4
    nc.compile()
    return nc


def _get_kernel():
    if "nc" not in _kernel_cache:
        _kernel_cache["nc"] = _build()
    return _kernel_cache["nc"]


def _prep_core(hbf, src_c, dst_c):
    """Per-core arrays.  src_c/dst_c are the core's E_CORE edges with
    dst_c sorted ascending.  Returns htab, src16, dstT, stab."""
    pe = E_PAD - len(src_c)
    s_p = np.concatenate([src_c, np.zeros(pe, src_c.dtype)])
    d_p = np.concatenate([dst_c, np.full(pe, dst_c[-1], dst_c.dtype)])

    htab = np.zeros((PHASES, TAB, H), hbf.dtype)
    src16 = np.zeros((P, NCHUNK * ICOLS), np.int16)
    pedges = CPP * CHUNK
    for ph in range(PHASES):
        ids = s_p[ph * pedges:(ph + 1) * pedges]
        uniq, inv = np.unique(ids, return_inverse=True)
        assert len(uniq) <= TAB, len(uniq)
        htab[ph, :len(uniq)] = hbf[uniq]
        blk = inv.astype(np.int16).reshape(CPP, ICOLS, 16)
        row16 = blk.transpose(2, 0, 1).reshape(16, CPP * ICOLS)
        src16[:, ph * CPP * ICOLS:(ph + 1) * CPP * ICOLS] = np.tile(row16, (8, 1))

    dstT = np.zeros((H, NCHUNK * P), hbf.dtype)
    stab = np.full((P, NCHUNK), CHUNK, np.float16)
    for ci in range(NCHUNK):
        d = d_p[ci * CHUNK:(ci + 1) * CHUNK]
        uniq, colid = np.unique(d, return_inverse=True)
        assert len(uniq) <= P, len(uniq)
        dstT[:, ci * P:ci * P + len(uniq)] = hbf[uniq].T
        stab[:, ci] = np.searchsorted(colid, np.arange(P)).astype(np.float16)
    return htab, src16, dstT, stab


def kernel(h, W, b, src, dst):
    import ml_dtypes
    from concourse.bass_utils import run_bass_kernel_spmd

    h = np.ascontiguousarray(np.asarray(h, dtype=np.float32))
    W = np.asarray(W, dtype=np.float32)
    b = np.asarray(b, dtype=np.float32)
    src = np.asarray(src)
    dst = np.asarray(dst)

    hbf = h.astype(ml_dtypes.bfloat16)
    # wt[k, c, f] = W[c, f, k], fp16 for the low-error diff chain
    wt = np.ascontiguousarray(W.transpose(2, 0, 1)).astype(np.float16)
    sel = np.zeros((P, SUPER * C, SUPER * C), np.float32)
    for r in range(SUPER * C):
        sel[:, r, r] = 1.0
    sel = sel.astype(ml_dtypes.bfloat16)
    bias = np.ascontiguousarray(
        np.tile(b[None, :], (SUPER, 1)).reshape(SUPER * C, 1)).astype(np.float32)
    iota = np.tile(np.arange(CHUNK, dtype=np.float16)[None, :], (P, 1))

    order = np.argsort(dst, kind="stable")
    in_maps = []
    for i in range(N_CORES):
        e_idx = order[i * E_CORE:(i + 1) * E_CORE]
        htab, src16, dstT, stab = _prep_core(hbf, src[e_idx], dst[e_idx])
        in_maps.append({
            "htab": htab, "dstt": dstT, "wt": wt, "sel": sel, "bias": bias,
            "iota": iota, "stab": stab, "srcx": src16,
        })

    nc = _get_kernel()
    _last_run["nc"] = nc
    _last_run["in_maps"] = in_maps
    res = run_bass_kernel_spmd(nc, in_maps, core_ids=list(range(N_CORES)))

    out = np.empty((E, C), np.float32)
    for i in range(N_CORES):
        sc = res.results[i]["scores"]              # [NCHUNK, C, CHUNK]
        slots = sc.transpose(0, 2, 1).reshape(E_PAD, C)
        out[order[i * E_CORE:(i + 1) * E_CORE]] = slots[:E_CORE]
    return out
